# revision 1
# baseline (speedup 1.0000x reference)
"""Trainium2 Bass kernel for the differentiable compressor.

Algorithm
---------
The time recurrence  s_t = a_t s_{t-1} + (1-a_t) v_t,
a_t = A_AT if v_t > s_{t-1} else A_REL  is a max-linear system:
    s_t = max(A_AT s + (1-A_AT) v,  A_REL s + (1-A_REL) v)
so policy iteration converges in a handful of sweeps: guess the modes m_t
from the current trajectory, solve the resulting *linear* recurrence exactly
with the hardware tensor_tensor_scan, repeat.  Empirically (fixed inputs,
key(0)) 4 lagged + 1 exact iteration reach ~1.6e-5 nat ~ 1.4e-4 dB.

Everything runs in natural-log units (v = ln(|x|+1e-8)); the dB scale
cancels through the whole pipeline.  The trajectory is kept relative to the
input, r_t = s_t - v_t, which turns the recurrence into
    r_t = a_t * (r_{t-1} + delta_t),   delta_t = v_{t-1} - v_t,
so the scan is  state = (delta add state) mult a  with delta PRECOMPUTED
once — no per-iteration right-hand-side pass — and the s0 = v init becomes
r0 = 0 (a memset).  Modes are  m_t = [r_{t-1} < -delta_t].

Layout per core: 2 batch rows x 441000 samples -> [126 partitions x 7000],
63 time-chunks per row.  Chunk-boundary carries live in an extra leading
column of the trajectory tile; lagged iterations seed each chunk's scan
with the previous iteration's end-of-previous-chunk state (two tiny
SBUF->SBUF DMAs); the final exact iteration solves chunks with zero
initials, computes true carries via chunk decay products (cumprod scan) +
a [2,63] cross-chunk mini-scan, and distributes them with a fixup pass.
The mode->coefficient affine runs on the Scalar engine in half-width
pieces so it hides under the Vector engine's compare/scan stream.

Sharding: pure data parallel, batch 16 -> 2 rows on each of 8 cores.
"""
import sys
import types
import numpy as np

# ---------------- constants (natural-log units) ----------------
SR = 44100.0
A_AT = float(np.exp(-1.0 / (10.0 * SR / 1000.0)))     # attack coeff
A_REL = float(np.exp(-1.0 / (100.0 * SR / 1000.0)))   # release coeff
DA = A_AT - A_REL
CNAT = float(np.log(10.0) / 20.0)                     # dB -> nat
KN = 0.1 * CNAT                                       # knee
EPS = 1e-4 * CNAT * CNAT
CDN = -(1.0 - 1.0 / 66.7) * 0.5                       # down-ratio gain slope
CUP = (1.0 - 0.1) * 0.5                               # up-ratio gain slope
UPR = 36.0 * CNAT                                     # up-range clamp
TMIN, TMAX = -40.0, 0.0

B, N = 16, 441000
NCORES = 8
ROWS = 2           # batch rows per core
NCH = 63           # chunks per row
P = ROWS * NCH     # 126 partitions
L = N // NCH       # 7000 chunk length
H = L // 2         # half-width for engine overlap

N_LAGGED = 4       # lagged policy iterations before the exact one


def _install_ntff_hook():
    """Inject the missing antenv.axon_hooks so trace=True profiling works."""
    try:
        import antenv
        if "antenv.axon_hooks" not in sys.modules:
            m = types.ModuleType("antenv.axon_hooks")
            m._hook = None
            def _set(h, _m=m): _m._hook = h
            def _get(_m=m): return _m._hook
            m.set_axon_ntff_profile_hook = _set
            m.get_axon_ntff_profile_hook = _get
            sys.modules["antenv.axon_hooks"] = m
            antenv.axon_hooks = m
            from trn_agent_boot.trn_boot import _ntff_profile_via_ctypes
            _set(_ntff_profile_via_ctypes("/opt/axon/libaxon_pjrt.so"))
    except Exception:
        pass


def build_nc():
    import concourse.bacc as bacc
    import concourse.mybir as mybir
    from concourse.tile import TileContext
    from concourse.alu_op_type import AluOpType as Op
    AF = mybir.ActivationFunctionType

    nc = bacc.Bacc("TRN2", target_bir_lowering=False, debug=False)
    x_d = nc.dram_tensor("x", [P, L], mybir.dt.float32, kind="ExternalInput")
    th_d = nc.dram_tensor("th", [P, 1], mybir.dt.float32, kind="ExternalInput")
    dep_d = nc.dram_tensor("dep", [P, 1], mybir.dt.float32, kind="ExternalInput")
    y_d = nc.dram_tensor("y", [P, L], mybir.dt.float32, kind="ExternalOutput")

    f32 = mybir.dt.float32
    with TileContext(nc) as tc:
        with tc.tile_pool(name="pool", bufs=1) as pool:
            tx = pool.tile([P, L], f32)        # x (resident; used at the end)
            tv = pool.tile([P, L], f32)        # v; post: dn-gate scratch
            tD = pool.tile([P, L], f32)        # delta; post: g, m1' scratch
            tse = pool.tile([P, L + 1], f32)   # r trajectory, col0 = boundary
            ta = pool.tile([P, L], f32)        # modes -> a; post: q2 scratch
            tth = pool.tile([P, 1], f32)
            tdep = pool.tile([P, 1], f32)
            tm2 = pool.tile([P, L], f32)       # post: up-gate mask
            te = pool.tile([P, 1], f32)        # exact carries
            tcol = pool.tile([P, 1], f32)      # prev-chunk-end v column
            tG2 = pool.tile([2, NCH], f32)     # transposed chunk decays
            tZ2 = pool.tile([2, NCH], f32)     # transposed chunk end-states
            teb = pool.tile([2, NCH + 1], f32) # mini-scan buffer
            # constant columns for activation bias operands
            tcst = pool.tile([P, 4], f32)
            c1e8, cKN, cEPS, cmKN = (tcst[:, i:i + 1] for i in range(4))
            nc.vector.memset(c1e8, 1e-8)
            nc.vector.memset(cKN, KN)
            nc.vector.memset(cEPS, EPS)
            nc.vector.memset(cmKN, -KN)

            W = tse[:, 1:L + 1]                # trajectory / later w view

            # v = ln(|x|+1e-8), chunked so the x DMA overlaps the ACT chain;
            # delta_t = v_{t-1} - v_t (th cancels within a partition) and
            # -delta.  delta[.,0] crosses chunks via a small column DMA; for
            # each row's first chunk v_{-1} := v_0 so delta = 0 there.
            NS = 8
            CW = L // NS
            nc.sync.dma_start(tth[:], th_d[:])
            nc.sync.dma_start(tdep[:], dep_d[:])
            for j in range(NS):
                sl = slice(j * CW, (j + 1) * CW)
                nc.sync.dma_start(tx[:, sl], x_d[:, sl])
            # r0 = 0 everywhere (s0 = v), including boundary column and the
            # permanent r_{-1}=0 of each row's chunk 0; teb col0 = 0 carry.
            nc.gpsimd.memset(tse[:], 0.0)
            nc.gpsimd.memset(teb[:, 0:1], 0.0)
            for j in range(NS):
                sl = slice(j * CW, (j + 1) * CW)
                nc.scalar.activation(tv[:, sl], tx[:, sl], AF.Abs, bias=0.0, scale=1.0)
                nc.scalar.activation(tv[:, sl], tv[:, sl], AF.Ln, bias=c1e8, scale=1.0)
                lo = j * CW
                s_in = slice(lo if j else 1, (j + 1) * CW)
                s_sh = slice((lo - 1) if j else 0, (j + 1) * CW - 1)
                nc.vector.tensor_tensor(tD[:, s_in], tv[:, s_sh], tv[:, s_in],
                                        Op.subtract)
            nc.sync.dma_start(tcol[1:NCH, 0:1], tv[0:NCH - 1, L - 1:L])
            nc.sync.dma_start(tcol[NCH + 1:P, 0:1], tv[NCH:P - 1, L - 1:L])
            nc.sync.dma_start(tcol[0:1, 0:1], tv[0:1, 0:1])
            nc.sync.dma_start(tcol[NCH:NCH + 1, 0:1], tv[NCH:NCH + 1, 0:1])
            nc.vector.tensor_tensor(tD[:, 0:1], tcol[:, 0:1], tv[:, 0:1], Op.subtract)

            for it in range(N_LAGGED + 1):
                final = it == N_LAGGED
                # modes: m_t = [r_{t-1} < -delta_t].  Column 0 fully split
                # out (its own tiny mode+coeff ops) so the big ops never
                # wait on the column-0 dependency chain (boundary DMA /
                # delta column).  Iteration 0 compares against r==0, a
                # cheaper tensor_scalar.  The a = m*DA + A_REL affine runs
                # on the Scalar engine, hidden under the m/scan stream.
                if it == 0:
                    nc.vector.tensor_scalar(ta[:, 1:H], tD[:, 1:H], 0.0, None,
                                            op0=Op.is_lt)
                else:
                    nc.vector.scalar_tensor_tensor(
                        ta[:, 1:H], tse[:, 1:H], -1.0, tD[:, 1:H],
                        op0=Op.mult, op1=Op.is_gt)
                nc.scalar.activation(ta[:, 1:H], ta[:, 1:H], AF.Copy,
                                     bias=A_REL, scale=DA)
                nc.vector.scalar_tensor_tensor(
                    ta[:, 0:1], tse[:, 0:1], -1.0, tD[:, 0:1],
                    op0=Op.mult, op1=Op.is_gt)
                nc.scalar.activation(ta[:, 0:1], ta[:, 0:1], AF.Copy,
                                     bias=A_REL, scale=DA)
                if it == 0:
                    nc.vector.tensor_scalar(ta[:, H:L], tD[:, H:L], 0.0, None,
                                            op0=Op.is_lt)
                else:
                    nc.vector.scalar_tensor_tensor(
                        ta[:, H:L], tse[:, H:L], -1.0, tD[:, H:L],
                        op0=Op.mult, op1=Op.is_gt)
                nc.scalar.activation(ta[:, H:L], ta[:, H:L], AF.Copy,
                                     bias=A_REL, scale=DA)

                if not final:
                    # lagged carry: initial = previous iterate's boundary col
                    nc.vector.tensor_tensor_scan(
                        tse[:, 1:H + 1], tD[:, 0:H], ta[:, 0:H], tse[:, 0:1],
                        op0=Op.add, op1=Op.mult)
                    nc.vector.tensor_tensor_scan(
                        tse[:, H + 1:L + 1], tD[:, H:L], ta[:, H:L], tse[:, H:H + 1],
                        op0=Op.add, op1=Op.mult)
                    # refresh boundary column from the new trajectory
                    nc.sync.dma_start(tse[1:NCH, 0:1], tse[0:NCH - 1, L:L + 1])
                    nc.sync.dma_start(tse[NCH + 1:P, 0:1], tse[NCH:P - 1, L:L + 1])
                else:
                    # exact: zero-init scans -> W holds z (chunk-local solves)
                    nc.vector.tensor_tensor_scan(
                        tse[:, 1:H + 1], tD[:, 0:H], ta[:, 0:H], 0.0,
                        op0=Op.add, op1=Op.mult)
                    nc.vector.tensor_tensor_scan(
                        tse[:, H + 1:L + 1], tD[:, H:L], ta[:, H:L], tse[:, H:H + 1],
                        op0=Op.add, op1=Op.mult)
                    # chunk end-states transfer while the cumprod scan runs
                    nc.sync.dma_start(tZ2[:], tse[:, L:L + 1])
                    # within-chunk decay cumprod g -> tD (delta is consumed);
                    # op1=bypass ignores data1
                    nc.vector.tensor_tensor_scan(
                        tD[:, 0:H], ta[:, 0:H], ta[:, 0:H], 1.0,
                        op0=Op.mult, op1=Op.bypass)
                    nc.vector.tensor_tensor_scan(
                        tD[:, H:L], ta[:, H:L], ta[:, H:L], tD[:, H - 1:H],
                        op0=Op.mult, op1=Op.bypass)
                    nc.sync.dma_start(tG2[:], tD[:, L - 1:L])
                    # w partial: W += v, overlapping the tG2/te DMA
                    # latency around the tiny cross-chunk mini-scan
                    nc.vector.tensor_tensor(tse[:, 1:H + 1], tse[:, 1:H + 1],
                                            tv[:, 0:H], Op.add)
                    # mini-scan: e[0]=0; e[c] = z0end[c-1] + G[c-1]*e[c-1]
                    nc.vector.tensor_tensor_scan(
                        teb[:, 1:NCH + 1], tG2[:], tZ2[:], 0.0,
                        op0=Op.mult, op1=Op.add)
                    # back to [P,1]: carry BEFORE chunk p = teb[., p]
                    nc.sync.dma_start(te[:], teb[:, 0:NCH])
                    nc.vector.tensor_tensor(tse[:, H + 1:L + 1], tse[:, H + 1:L + 1],
                                            tv[:, H:L], Op.add)
                    # fixup: w = z + v + (g*e - th).  Halved: W's first half
                    # finalizes early so the gain post-processing (ACT
                    # square/sqrt chains) starts while DVE finishes h1.
                    nc.vector.tensor_scalar(tD[:], tD[:], te[:, 0:1], tth[:, 0:1],
                                            op0=Op.mult, op1=Op.subtract)
                    for h in range(2):
                        sl = slice(h * H, (h + 1) * H)
                        slW = slice(h * H + 1, (h + 1) * H + 1)
                        nc.vector.tensor_tensor(tse[:, slW], tse[:, slW],
                                                tD[:, sl], Op.add)

            # ---------------- gain computation ----------------
            # W holds w = s - th = diff_dn.  Per half: gate masks with the
            # gain slopes folded in (DVE), q1/q2 square+sqrt chains (ACT,
            # grouped so the function-table switches stay rare), combines
            # (DVE), then the Exp/multiply/store tail pipelines across
            # ACT / DVE / DMA.
            #   m1' = CDN*[w > -KN]   (down gate)
            #   m2' = CUP*[w < KN]    (up gate)
            #   gdn = (q1 + KN + w) * m1',  q1 = sqrt((w-KN)^2 + EPS)
            #   gup = min((q2 + KN - w) * m2', UPR), q2 = sqrt((w+KN)^2+EPS)
            for h in range(2):
                sl = slice(h * H, (h + 1) * H)
                Wh = tse[:, h * H + 1:(h + 1) * H + 1]
                nc.scalar.activation(tD[:, sl], Wh, AF.Square, bias=cmKN, scale=1.0)
                nc.scalar.activation(ta[:, sl], Wh, AF.Square, bias=cKN, scale=1.0)
                nc.scalar.activation(tD[:, sl], tD[:, sl], AF.Sqrt, bias=cEPS, scale=1.0)
                nc.scalar.activation(ta[:, sl], ta[:, sl], AF.Sqrt, bias=cEPS, scale=1.0)
                nc.vector.tensor_scalar(tv[:, sl], Wh, -KN, CDN, op0=Op.is_gt, op1=Op.mult)
                nc.vector.tensor_scalar(tm2[:, sl], Wh, KN, CUP, op0=Op.is_lt, op1=Op.mult)
                nc.vector.scalar_tensor_tensor(
                    tD[:, sl], tD[:, sl], KN, Wh, op0=Op.add, op1=Op.add)
                nc.vector.tensor_tensor(tD[:, sl], tD[:, sl], tv[:, sl], Op.mult)
                nc.vector.scalar_tensor_tensor(
                    ta[:, sl], ta[:, sl], KN, Wh, op0=Op.add, op1=Op.subtract)
                nc.vector.tensor_tensor(ta[:, sl], ta[:, sl], tm2[:, sl], Op.mult)
                nc.vector.tensor_scalar(ta[:, sl], ta[:, sl], UPR, None, op0=Op.min)
                nc.vector.tensor_tensor(tD[:, sl], tD[:, sl], ta[:, sl], Op.add)
                for q in range(2):
                    sq = slice(h * H + q * (H // 2), h * H + (q + 1) * (H // 2))
                    nc.scalar.activation(tD[:, sq], tD[:, sq], AF.Exp,
                                         bias=0.0, scale=tdep[:, 0:1])
                    nc.vector.tensor_tensor(ta[:, sq], tD[:, sq], tx[:, sq], Op.mult)
                    nc.sync.dma_start(y_d[:, sq], ta[:, sq])

    nc.compile()
    return nc


_NC = None


def _get_nc():
    global _NC
    if _NC is None:
        _NC = build_nc()
    return _NC


def make_in_maps(x, threshold, depth):
    th_nat = ((TMIN + threshold.astype(np.float32) * (TMAX - TMIN)) *
              np.float32(CNAT)).astype(np.float32)           # [16,1]
    dep = depth.astype(np.float32)
    in_maps = []
    for i in range(NCORES):
        xs = np.ascontiguousarray(x[ROWS * i:ROWS * (i + 1)]).reshape(P, L)
        ths = np.repeat(th_nat[ROWS * i:ROWS * (i + 1), 0], NCH).reshape(P, 1)
        deps = np.repeat(dep[ROWS * i:ROWS * (i + 1), 0], NCH).reshape(P, 1)
        in_maps.append({"x": xs.astype(np.float32),
                        "th": np.ascontiguousarray(ths, np.float32),
                        "dep": np.ascontiguousarray(deps, np.float32)})
    return in_maps


def kernel(x, threshold, depth):
    _install_ntff_hook()
    from concourse.bass_utils import run_bass_kernel_spmd
    nc = _get_nc()
    x = np.asarray(x, np.float32)
    in_maps = make_in_maps(x, np.asarray(threshold), np.asarray(depth))
    res = run_bass_kernel_spmd(nc, in_maps, core_ids=list(range(NCORES)))
    y = np.empty((B, N), np.float32)
    for i in range(NCORES):
        y[ROWS * i:ROWS * (i + 1)] = np.asarray(res.results[i]["y"]).reshape(ROWS, N)
    return y

